# revision 19
# baseline (speedup 1.0000x reference)
"""Bass/Trainium2 kernel for nn_BinaryLSTMCell (B=65536, D=U=256).

Strategy (data-parallel over 8 cores, 8192 batch rows each):
  - Host: binarize kernels, permute recurrent kernel columns so the
    r-projection accumulates gate-aligned with the x-projection
    (reference pairs f<-x_i+r_f, i<-x_f+r_i), pack inputs/h transposed
    so every device DMA is a flat [128, 2048] contiguous copy.
  - Device, per 128-row tile: x-matmuls -> PSUM, hard-tanh clamp of the
    PSUM in place (DVE), r-matmuls accumulate on top (PE has_written
    bits stay set from the x-matmuls so accumulation lands on the
    clamped values), ACT evicts pre-gates to SBUF with cast, GPSIMD
    clamps the gates, DVE runs the c/h elementwise chain slab-wise.
"""

import os
import sys

for _p in ("/opt/trn_rl_repo", "/root/.axon_site/_ro/trn_rl_repo"):
    if os.path.isdir(_p) and _p not in sys.path:
        sys.path.append(_p)

import numpy as np
from contextlib import ExitStack

import concourse.bass as bass
import concourse.bacc as bacc
import concourse.mybir as mybir
from concourse.tile import TileContext
from concourse.bass_utils import run_bass_kernel_spmd

F32 = mybir.dt.float32
F32R = mybir.dt.float32r
BF16 = mybir.dt.bfloat16
ALU = mybir.AluOpType

N_CORES = 8
B = 65536
D = 256
U = 256
B_CORE = B // N_CORES          # 8192
SUPER = 1024                   # batch rows per super-tile
T_PER_S = SUPER // 128         # 8 tiles of 128 rows per super-tile
N_SUPER = B_CORE // SUPER      # 8 super-tiles per core

# gate/intermediate dtype: bf16 keeps DVE in 2x/4x perf modes.
GATE_DT = BF16


def _clamp(eng, out_ap, in_ap):
    """out = min(max(in, -1), 1) as one fused tensor_scalar."""
    eng.tensor_scalar(out_ap, in_ap, -1.0, 1.0, ALU.max, ALU.min)


def build_program(n_super=N_SUPER, gate_dt=GATE_DT):
    ABL = os.environ.get("KABL", "")
    """Build the per-core SPMD Bass program.

    DRAM layouts (all per-core):
      x, h   : [n_super, 128, 2048]  packed transposed activations;
               free index = k*1024 + t*128 + q holds element
               [d = k*128 + p, batch row = s*1024 + 8*q + t]
      c      : [n_super, 128, 2048]  natural rows; partition q, free
               t*256 + u holds c[s*1024 + 8*q + t, u]
      wx, wr : [128, 2048] binarized weights; free = k*1024 + col
      hn, cn : like c (outputs)
    """
    nc = bacc.Bacc("TRN2", target_bir_lowering=False, debug=False)

    x_d = nc.dram_tensor("x", [n_super, 128, 2048], F32R, kind="ExternalInput")
    h_d = nc.dram_tensor("h", [n_super, 128, 2048], F32R, kind="ExternalInput")
    c_d = nc.dram_tensor("c", [n_super, 128, 2048], F32, kind="ExternalInput")
    wx_d = nc.dram_tensor("wx", [128, 2048], F32R, kind="ExternalInput")
    wr_d = nc.dram_tensor("wr", [128, 2048], F32R, kind="ExternalInput")
    hn_d = nc.dram_tensor("hn", [n_super, 128, 2048], F32, kind="ExternalOutput")
    cn_d = nc.dram_tensor("cn", [n_super, 128, 2048], F32, kind="ExternalOutput")

    with TileContext(nc) as tc, ExitStack() as ctx:
        wpool = ctx.enter_context(tc.tile_pool(name="w", bufs=1))
        iopool = ctx.enter_context(tc.tile_pool(name="io", bufs=2))
        inpool = ctx.enter_context(tc.tile_pool(name="inp", bufs=int(os.environ.get("KINBUFS", "3"))))
        mpool = ctx.enter_context(tc.tile_pool(name="mid", bufs=int(os.environ.get("KMIDBUFS", "2"))))
        pspool = ctx.enter_context(tc.tile_pool(name="ps", bufs=4, space="PSUM"))

        wx = wpool.tile([128, 2048], F32R, tag="wx")
        nc.sync.dma_start(wx[:], wx_d.ap()[:, :])
        wr = wpool.tile([128, 2048], F32R, tag="wr")
        nc.sync.dma_start(wr[:], wr_d.ap()[:, :])

        def stage_in(s):
            xt = inpool.tile([128, 2048], F32R, tag="xt", name=f"xt_{s}")
            nc.sync.dma_start(xt[:], x_d.ap()[s])
            ht = inpool.tile([128, 2048], F32R, tag="ht", name=f"ht_{s}")
            nc.sync.dma_start(ht[:], h_d.ap()[s])
            ct = iopool.tile([128, 2048], F32, tag="ct", name=f"ct_{s}")
            nc.sync.dma_start(ct[:], c_d.ap()[s])
            # h_tm1 = hard_tanh(h): in place, feeds the r matmuls.
            _clamp(nc.vector, ht[:], ht[:])
            # c_tm1 = hard_tanh(c) (cast to gate dtype)
            cc = mpool.tile([128, 2048], gate_dt, tag="cc", name=f"cc_{s}")
            _clamp(nc.gpsimd, cc[:], ct[:])
            return xt, ht, cc

        staged = {}
        for s in range(n_super):
            if s not in staged:
                staged[s] = stage_in(s)
            xt, ht, cc = staged.pop(s)

            gates = mpool.tile([128, T_PER_S, 1024], gate_dt, tag="gates")

            # float32r runs the PE at full rate (1 cyc/row vs 4 for fp32
            # when the moving dim >= 256); bits are fp32, PE rounds
            # internally. Operands are bitcast views; PSUM stays fp32.
            def emit_xproj(t, ps):
                for n in range(2):
                    o = ps[:, n * 512:(n + 1) * 512]
                    nc.tensor.matmul(
                        o,
                        (xt[:, t * 128:(t + 1) * 128]),
                        (wx[:, n * 512:(n + 1) * 512]),
                        start=True, stop=False,
                    )
                    nc.tensor.matmul(
                        o,
                        (xt[:, 1024 + t * 128:1024 + (t + 1) * 128]),
                        (wx[:, 1024 + n * 512:1024 + (n + 1) * 512]),
                        start=False, stop=True,
                    )

            def emit_rproj(t, ps):
                for n in range(2):
                    o = ps[:, n * 512:(n + 1) * 512]
                    nc.tensor.matmul(
                        o,
                        (ht[:, t * 128:(t + 1) * 128]),
                        (wr[:, n * 512:(n + 1) * 512]),
                        start=False, stop=False, skip_group_check=True,
                    )
                    nc.tensor.matmul(
                        o,
                        (ht[:, 1024 + t * 128:1024 + (t + 1) * 128]),
                        (wr[:, 1024 + n * 512:1024 + (n + 1) * 512]),
                        start=False, stop=(n == 1), skip_group_check=True,
                    )

            # Software-pipelined across tiles so PE never stalls on the DVE
            # PSUM clamp: x-MMs(t+1) run while clamp(t) happens; per-engine
            # execution follows emission order.
            pss = {}
            for t in range(T_PER_S + 1):
                if t < T_PER_S:
                    pss[t] = pspool.tile([128, 1024], F32, tag="ps",
                                         name=f"ps_{s}_{t}")
                    emit_xproj(t, pss[t])
                    _clamp(nc.vector, pss[t][:], pss[t][:])
                if t >= 1:
                    tp = t - 1
                    emit_rproj(tp, pss[tp])
                    # evict pre-gates (cast) then clamp -> f,i,g,o
                    nc.scalar.copy(gates[:, tp, :], pss[tp][:])
                    if os.environ.get("KGCLAMP", "dve") == "dve":
                        _clamp(nc.vector, gates[:, tp, :], gates[:, tp, :])
                    else:
                        _clamp(nc.gpsimd, gates[:, tp, :], gates[:, tp, :])
                    del pss[tp]

            # prefetch + pre-clamp next super-tile before the tail chain
            # so PE's next r-matmuls never wait on the DVE h-clamp
            if s + 1 < n_super and os.environ.get("KPREF", "1") == "1":
                staged[s + 1] = stage_in(s + 1)

            # slab-wise elementwise; 3-D APs keep the free dim contiguous
            def g3(lo, hi):
                return gates[:, :, lo:hi]

            def s3(tile):
                return tile[:].rearrange("p (t u) -> p t u", u=256)

            t1 = mpool.tile([128, 2048], gate_dt, tag="t1")
            t2 = mpool.tile([128, 2048], gate_dt, tag="t2")
            z = mpool.tile([128, 2048], gate_dt, tag="z")
            cnew = iopool.tile([128, 2048], F32, tag="cn")
            hnew = iopool.tile([128, 2048], F32, tag="hn")

            if ABL != "noelem":
                tteng = nc.gpsimd if os.environ.get("KTT", "dve") == "pool" else nc.vector
                zeng = nc.vector if os.environ.get("KZ", "pool") == "dve" else nc.gpsimd
                halves = int(os.environ.get("KHALVES", "1"))
                tp = T_PER_S // halves
                for hh in range(halves):
                    ts_ = slice(hh * tp, (hh + 1) * tp)
                    cs = slice(hh * tp * 256, (hh + 1) * tp * 256)
                    tteng.tensor_tensor(s3(t1)[:, ts_], g3(0, 256)[:, ts_], s3(cc)[:, ts_], ALU.mult)
                    tteng.tensor_tensor(s3(t2)[:, ts_], g3(256, 512)[:, ts_], g3(512, 768)[:, ts_], ALU.mult)
                    nc.vector.tensor_tensor(s3(cnew)[:, ts_], s3(t1)[:, ts_], s3(t2)[:, ts_], ALU.add)
                    _clamp(zeng, z[:, cs], cnew[:, cs])
                    # o, z in [-1,1] so the outer hard_tanh is the identity
                    nc.vector.tensor_tensor(s3(hnew)[:, ts_], g3(768, 1024)[:, ts_], s3(z)[:, ts_], ALU.mult)
            else:
                nc.vector.tensor_copy(s3(cnew), g3(0, 256))
                nc.vector.tensor_copy(s3(hnew), g3(768, 1024))

            nc.sync.dma_start(hn_d.ap()[s], hnew[:])
            nc.sync.dma_start(cn_d.ap()[s], cnew[:])

    nc.compile()
    return nc


def _pack_activation(a_core):
    """[rows, 256] -> [n_super, 128, 2048] transposed+permuted layout."""
    n_super = a_core.shape[0] // SUPER
    # [s, q, t, k, p] from rows s*1024 + 8q + t, cols k*128 + p
    v = a_core.reshape(n_super, 128, 8, 2, 128)
    return np.ascontiguousarray(v.transpose(0, 4, 3, 2, 1)).reshape(
        n_super, 128, 2048)


def _pack_weight(w):
    """[256, 1024] -> [128, 2048] with free = k*1024 + col."""
    return np.ascontiguousarray(
        w.reshape(2, 128, 1024).transpose(1, 0, 2)).reshape(128, 2048)


_PROGRAM_CACHE = {}


def _get_program():
    key = (N_SUPER, GATE_DT)
    if key not in _PROGRAM_CACHE:
        _PROGRAM_CACHE[key] = build_program()
    return _PROGRAM_CACHE[key]


def _run(inputs, h, c, kernel_w, recurrent_kernel, trace=False):
    X = np.ascontiguousarray(np.asarray(inputs, dtype=np.float32))
    H = np.ascontiguousarray(np.asarray(h, dtype=np.float32))
    C = np.ascontiguousarray(np.asarray(c, dtype=np.float32))
    Wk = np.asarray(kernel_w, dtype=np.float32)
    Rk = np.asarray(recurrent_kernel, dtype=np.float32)

    Wb = np.where(Wk >= 0, np.float32(1.0), np.float32(-1.0))
    Rb = np.where(Rk >= 0, np.float32(1.0), np.float32(-1.0))
    # reorder r columns to [r_f, r_i, r_c, r_o] so PSUM accumulation is
    # gate-aligned (f pairs x_i with W_f, i pairs x_f with W_i)
    Rb = np.concatenate(
        [Rb[:, U:2 * U], Rb[:, 0:U], Rb[:, 2 * U:3 * U], Rb[:, 3 * U:]], axis=1)

    wx_np = _pack_weight(Wb)
    wr_np = _pack_weight(Rb)

    in_maps = []
    for m in range(N_CORES):
        lo, hi = m * B_CORE, (m + 1) * B_CORE
        in_maps.append({
            "x": _pack_activation(X[lo:hi]),
            "h": _pack_activation(H[lo:hi]),
            "c": np.ascontiguousarray(C[lo:hi]).reshape(N_SUPER, 128, 2048),
            "wx": wx_np,
            "wr": wr_np,
        })

    nc = _get_program()
    res = run_bass_kernel_spmd(nc, in_maps, core_ids=list(range(N_CORES)),
                               trace=trace)

    h_new = np.empty((B, U), dtype=np.float32)
    c_new = np.empty((B, U), dtype=np.float32)
    for m in range(N_CORES):
        lo, hi = m * B_CORE, (m + 1) * B_CORE
        h_new[lo:hi] = res.results[m]["hn"].reshape(B_CORE, U)
        c_new[lo:hi] = res.results[m]["cn"].reshape(B_CORE, U)
    return (h_new, h_new, c_new), res


def kernel(inputs, h, c, kernel, recurrent_kernel):
    outs, _ = _run(inputs, h, c, kernel, recurrent_kernel, trace=False)
    return outs
